# revision 9
# baseline (speedup 1.0000x reference)
"""Self-attention (CrossAttention with context=x) Trainium2 Bass kernel.

Sharding: B*h = 16 head-instances across 8 cores -> each core owns one batch
and 2 heads. Per core (all on device): QKV projections (fp32r matmuls from
x^T), S^T = K Q^T per head (fp32r), exp on ScalarE (scale=1/8 folded, bf16
out), PV matmul with ones-augmented V (softmax denominators for free),
output projection per head + per-query-row 1/sum via tensor_scalar.
Host: transpose x per batch, slice per-head weight columns, sum the 4
per-batch partial outputs, add bias.
"""
import sys
sys.path.insert(0, "/opt/trn_rl_repo")

import numpy as np
from contextlib import ExitStack

import concourse.bass as bass
import concourse.tile as tile
from concourse import bacc, mybir
from concourse import bass_utils

DH = 64
D = 512
SEQ = 4096
B = 2
N_CORES = 8

f32 = mybir.dt.float32
f32r = mybir.dt.float32r
bf16 = mybir.dt.bfloat16
Exp = mybir.ActivationFunctionType.Exp


def build_nc(N=SEQ):
    nc = bacc.Bacc("TRN2", target_bir_lowering=False, debug=False,
                   num_devices=N_CORES)
    xT = nc.dram_tensor("xT", [D, N], bf16, kind="ExternalInput").ap()
    Wq2 = nc.dram_tensor("Wq2", [D, 128], bf16, kind="ExternalInput").ap()
    Wk2 = nc.dram_tensor("Wk2", [D, 128], bf16, kind="ExternalInput").ap()
    Wv2 = nc.dram_tensor("Wv2", [D, 128], bf16, kind="ExternalInput").ap()
    Wo2 = nc.dram_tensor("Wo2", [128, D], bf16, kind="ExternalInput").ap()
    Y = nc.dram_tensor("Y", [N, D], f32, kind="ExternalOutput").ap()

    KC = N // 128            # number of 128-key chunks
    QB = min(1024, N)        # query block
    NQB = N // QB
    QC = QB // 512           # 512-query chunks per block
    MT = QB // 128           # 128-query tiles per block
    ND = D // 128

    with tile.TileContext(nc) as tc, ExitStack() as ctx:
        wp = ctx.enter_context(tc.tile_pool(name="weights", bufs=1))
        pp = ctx.enter_context(tc.tile_pool(name="persist", bufs=1))
        qT = pp.tile([128, N], bf16, tag="qT")     # rows 0-63 head A, 64-127 head B
        kT = pp.tile([128, N], bf16, tag="kT")
        VA = pp.tile([128, KC * 65], bf16, tag="VA")   # [keys, 64 V + ones]
        VB = pp.tile([128, KC * 65], bf16, tag="VB")
        OTa = pp.tile([64, N], bf16, tag="OTa")    # head A O^T
        OTb = pp.tile([64, N], bf16, tag="OTb")
        scolA = pp.tile([128, N // 128], f32, tag="scolA")
        scolB = pp.tile([128, N // 128], f32, tag="scolB")
        rcolA = pp.tile([128, N // 128], f32, tag="rcolA")
        rcolB = pp.tile([128, N // 128], f32, tag="rcolB")

        wq = wp.tile([128, ND, 128], bf16, tag="wq")
        wk = wp.tile([128, ND, 128], bf16, tag="wk")
        wv = wp.tile([128, ND, 128], bf16, tag="wv")
        wo = wp.tile([64, 2, D], bf16, tag="wo")
        nc.sync.dma_start(wq[:], Wq2.rearrange("(t p) m -> p t m", p=128))
        nc.sync.dma_start(wk[:], Wk2.rearrange("(t p) m -> p t m", p=128))
        nc.sync.dma_start(wv[:], Wv2.rearrange("(t p) m -> p t m", p=128))
        nc.sync.dma_start(wo[:], Wo2.rearrange("(h p) d -> p h d", p=64))

        # ones columns of V_aug
        nc.vector.memset(VA[:, 64::65], 1.0)
        nc.vector.memset(VB[:, 64::65], 1.0)

        # ---- prefix: load x^T, compute qT/kT/V ----
        with tc.tile_pool(name="xload", bufs=1) as xp, \
             tc.tile_pool(name="prefps", bufs=2, space="PSUM") as prefps:
            xTd = xT.rearrange("(t p) n -> p t n", p=128)
            xts = []
            for t in range(ND):
                xt_t = xp.tile([128, N], bf16, tag=f"x{t}")
                nc.sync.dma_start(xt_t[:], xTd[:, t, :])
                xts.append(xt_t)

            class _XV:
                def __getitem__(self, idx):
                    _, t, cols = idx
                    return xts[t][:, cols]
            xTs = _XV()

            for qc in range(N // 512):
                ps = prefps.tile([128, 512], f32, tag="pq")
                for d in range(ND):
                    nc.tensor.matmul(ps[:], wq[:, d, :], xTs[:, d, qc * 512:(qc + 1) * 512],
                                     start=(d == 0), stop=(d == ND - 1))
                nc.vector.tensor_copy(qT[:, qc * 512:(qc + 1) * 512], ps[:])
                ps2 = prefps.tile([128, 512], f32, tag="pq")
                for d in range(ND):
                    nc.tensor.matmul(ps2[:], wk[:, d, :], xTs[:, d, qc * 512:(qc + 1) * 512],
                                     start=(d == 0), stop=(d == ND - 1))
                nc.vector.tensor_copy(kT[:, qc * 512:(qc + 1) * 512], ps2[:])
            for kc in range(KC):
                psv = prefps.tile([128, 128], f32, tag="pv")
                for d in range(ND):
                    nc.tensor.matmul(psv[:], xTs[:, d, kc * 128:(kc + 1) * 128], wv[:, d, :],
                                     start=(d == 0), stop=(d == ND - 1))
                nc.vector.tensor_copy(VA[:, kc * 65:kc * 65 + 64], psv[:, 0:64])
                nc.vector.tensor_copy(VB[:, kc * 65:kc * 65 + 64], psv[:, 64:128])

        # ---- attention + projection, pipelined over (qb, h) ----
        with tc.tile_pool(name="pt", bufs=KC // 2 + 2) as ptp, \
             tc.tile_pool(name="srowp", bufs=2) as srp, \
             tc.tile_pool(name="ysb", bufs=4) as yp, \
             tc.tile_pool(name="dramscratch", bufs=2, space="DRAM") as dp, \
             tc.tile_pool(name="stps", bufs=1, space="PSUM") as stps, \
             tc.tile_pool(name="otps", bufs=2, space="PSUM") as otps, \
             tc.tile_pool(name="yps", bufs=2, space="PSUM") as yps:

            for qb in range(NQB):
                q0 = qb * QB
                for h in range(2):
                    hp = 64 * h
                    kTh = kT
                    pts = []
                    for kp in range(KC // 2):
                        st = stps.tile([128, 2 * QB], f32, tag="st")
                        for half in range(2):
                            kc = 2 * kp + half
                            for j in range(QC):
                                o = half * QB + j * 512
                                nc.tensor.matmul(
                                    st[:, o:o + 512],
                                    kT[hp:hp + 64, kc * 128:(kc + 1) * 128],
                                    qT[hp:hp + 64, q0 + j * 512:q0 + (j + 1) * 512],
                                    start=True, stop=True)
                        pt = ptp.tile([128, 2 * QB], bf16, tag="pt")
                        nc.scalar.activation(pt[:], st[:], Exp, scale=0.125)
                        pts.append(pt)
                    Vh = VA if h == 0 else VB
                    OTh = OTa if h == 0 else OTb
                    srow = srp.tile([65, QB], f32, tag="srow")
                    for j in range(QC):
                        po = otps.tile([65, 512], f32, tag="ot")
                        for kc in range(KC):
                            src_pt = pts[kc // 2]
                            o = (kc % 2) * QB + j * 512
                            nc.tensor.matmul(po[:], Vh[:, kc * 65:(kc + 1) * 65],
                                             src_pt[:, o:o + 512],
                                             start=(kc == 0), stop=(kc == KC - 1))
                        nc.vector.tensor_copy(
                            OTh[:, q0 + j * 512:q0 + (j + 1) * 512], po[0:64, :])
                        nc.vector.tensor_copy(srow[64:65, j * 512:(j + 1) * 512],
                                              po[64:65, :])
                    # sums -> per-partition layout via DRAM round trip
                    srd = dp.tile([1, QB], f32, tag="srd")
                    nc.sync.dma_start(srd[:], srow[64:65, :])
                    scol = scolA if h == 0 else scolB
                    rcol = rcolA if h == 0 else rcolB
                    nc.sync.dma_start(
                        scol[:, qb * MT:(qb + 1) * MT],
                        srd.rearrange("a (m p) -> (a p) m", p=128))
                    nc.vector.reciprocal(rcol[:, qb * MT:(qb + 1) * MT],
                                         scol[:, qb * MT:(qb + 1) * MT])
                # projection for this query block (both heads ready)
                for m in range(MT):
                    mg = qb * MT + m
                    c0 = q0 + m * 128
                    pa = yps.tile([128, 512], f32, tag="ya")
                    nc.tensor.matmul(pa[:], OTa[:, c0:c0 + 128], wo[:, 0, :],
                                     start=True, stop=True)
                    ya = yp.tile([128, 512], f32, tag="ysa")
                    nc.vector.tensor_scalar_mul(ya[:], pa[:], rcolA[:, mg:mg + 1])
                    pb = yps.tile([128, 512], f32, tag="ya")
                    nc.tensor.matmul(pb[:], OTb[:, c0:c0 + 128], wo[:, 1, :],
                                     start=True, stop=True)
                    yb = yp.tile([128, 512], f32, tag="ysb")
                    nc.vector.tensor_scalar_mul(yb[:], pb[:], rcolB[:, mg:mg + 1])
                    yo = yp.tile([128, 512], f32, tag="yso")
                    nc.vector.tensor_add(yo[:], ya[:], yb[:])
                    nc.sync.dma_start(Y[c0:c0 + 128, :], yo[:])
    nc.compile()
    return nc


_NC_CACHE = {}


def _get_nc(N=SEQ):
    if N not in _NC_CACHE:
        _NC_CACHE[N] = build_nc(N)
    return _NC_CACHE[N]


def kernel(x, Wq, Wk, Wv, Wo, bo):
    x = np.asarray(x, dtype=np.float32)
    Wq = np.asarray(Wq, dtype=np.float32)
    Wk = np.asarray(Wk, dtype=np.float32)
    Wv = np.asarray(Wv, dtype=np.float32)
    Wo = np.asarray(Wo, dtype=np.float32)
    bo = np.asarray(bo, dtype=np.float32)
    Bx, N, Dx = x.shape
    nc = _get_nc(N)
    in_maps = []
    import ml_dtypes
    bfl = ml_dtypes.bfloat16
    xTs = [np.ascontiguousarray(x[b].T).astype(bfl) for b in range(Bx)]
    for c in range(N_CORES):
        b = c // 4
        hA = 2 * (c % 4)
        cols = slice(hA * DH, (hA + 2) * DH)
        in_maps.append({
            "xT": xTs[b],
            "Wq2": np.ascontiguousarray(Wq[:, cols]).astype(bfl),
            "Wk2": np.ascontiguousarray(Wk[:, cols]).astype(bfl),
            "Wv2": np.ascontiguousarray(Wv[:, cols]).astype(bfl),
            "Wo2": np.ascontiguousarray(Wo[cols, :]).astype(bfl),
        })
    res = bass_utils.run_bass_kernel_spmd(nc, in_maps, core_ids=list(range(N_CORES)))
    out = np.zeros((Bx, N, Dx), dtype=np.float32)
    for c in range(N_CORES):
        out[c // 4] += res.results[c]["Y"]
    out += bo
    return out


# revision 10
# speedup vs baseline: 1.4572x; 1.4572x over previous
"""Self-attention (CrossAttention with context=x) Trainium2 Bass kernel.

Sharding: B*h = 16 head-instances across 8 cores -> each core owns one batch
and 2 heads. Per core (all on device): QKV projections (fp32r matmuls from
x^T), S^T = K Q^T per head (fp32r), exp on ScalarE (scale=1/8 folded, bf16
out), PV matmul with ones-augmented V (softmax denominators for free),
output projection per head + per-query-row 1/sum via tensor_scalar.
Host: transpose x per batch, slice per-head weight columns, sum the 4
per-batch partial outputs, add bias.
"""
import sys
sys.path.insert(0, "/opt/trn_rl_repo")

import numpy as np
from contextlib import ExitStack

import concourse.bass as bass
import concourse.tile as tile
from concourse import bacc, mybir
from concourse import bass_utils

DH = 64
D = 512
SEQ = 4096
B = 2
N_CORES = 8

f32 = mybir.dt.float32
f32r = mybir.dt.float32r
bf16 = mybir.dt.bfloat16
Exp = mybir.ActivationFunctionType.Exp


def build_nc(N=SEQ):
    nc = bacc.Bacc("TRN2", target_bir_lowering=False, debug=False,
                   num_devices=N_CORES)
    xT = nc.dram_tensor("xT", [D, N], bf16, kind="ExternalInput").ap()
    Wq2 = nc.dram_tensor("Wq2", [D, 128], bf16, kind="ExternalInput").ap()
    Wk2 = nc.dram_tensor("Wk2", [D, 128], bf16, kind="ExternalInput").ap()
    Wv2 = nc.dram_tensor("Wv2", [D, 128], bf16, kind="ExternalInput").ap()
    Wo2 = nc.dram_tensor("Wo2", [128, D], bf16, kind="ExternalInput").ap()
    Y = nc.dram_tensor("Y", [N, D], f32, kind="ExternalOutput").ap()

    KC = N // 128            # number of 128-key chunks
    QB = min(1024, N)        # query block
    NQB = N // QB
    QC = QB // 512           # 512-query chunks per block
    MT = QB // 128           # 128-query tiles per block
    ND = D // 128

    with tile.TileContext(nc) as tc, ExitStack() as ctx:
        wp = ctx.enter_context(tc.tile_pool(name="weights", bufs=1))
        pp = ctx.enter_context(tc.tile_pool(name="persist", bufs=1))
        qT = pp.tile([128, N], bf16, tag="qT")     # rows 0-63 head A, 64-127 head B
        kT = pp.tile([128, N], bf16, tag="kT")
        VA = pp.tile([128, KC * 65], bf16, tag="VA")   # [keys, 64 V + ones]
        VB = pp.tile([128, KC * 65], bf16, tag="VB")
        OTa = pp.tile([64, N], bf16, tag="OTa")    # head A O^T
        OTb = pp.tile([64, N], bf16, tag="OTb")
        scolA = pp.tile([128, N // 128], f32, tag="scolA")
        scolB = pp.tile([128, N // 128], f32, tag="scolB")
        rcolA = pp.tile([128, N // 128], f32, tag="rcolA")
        rcolB = pp.tile([128, N // 128], f32, tag="rcolB")

        wq = wp.tile([128, ND, 128], bf16, tag="wq")
        wk = wp.tile([128, ND, 128], bf16, tag="wk")
        wv = wp.tile([128, ND, 128], bf16, tag="wv")
        wo = wp.tile([64, 2, D], bf16, tag="wo")
        nc.sync.dma_start(wq[:], Wq2.rearrange("(t p) m -> p t m", p=128))
        nc.sync.dma_start(wk[:], Wk2.rearrange("(t p) m -> p t m", p=128))
        nc.sync.dma_start(wv[:], Wv2.rearrange("(t p) m -> p t m", p=128))
        nc.sync.dma_start(wo[:], Wo2.rearrange("(h p) d -> p h d", p=64))

        # ones columns of V_aug
        nc.vector.memset(VA[:, 64::65], 1.0)
        nc.vector.memset(VB[:, 64::65], 1.0)

        # ---- prefix: load x^T, compute qT/kT/V ----
        with tc.tile_pool(name="xload", bufs=1) as xp, \
             tc.tile_pool(name="prefps", bufs=2, space="PSUM") as prefps:
            xTd = xT.rearrange("(t p) n -> p t n", p=128)
            xts = []
            for t in range(ND):
                xt_t = xp.tile([128, N], bf16, tag=f"x{t}")
                nc.sync.dma_start(xt_t[:], xTd[:, t, :])
                xts.append(xt_t)

            class _XV:
                def __getitem__(self, idx):
                    _, t, cols = idx
                    return xts[t][:, cols]
            xTs = _XV()

            for qc in range(N // 512):
                ps = prefps.tile([128, 512], f32, tag="pq")
                for d in range(ND):
                    nc.tensor.matmul(ps[:], wq[:, d, :], xTs[:, d, qc * 512:(qc + 1) * 512],
                                     start=(d == 0), stop=(d == ND - 1))
                nc.vector.tensor_copy(qT[:, qc * 512:(qc + 1) * 512], ps[:])
                ps2 = prefps.tile([128, 512], f32, tag="pq")
                for d in range(ND):
                    nc.tensor.matmul(ps2[:], wk[:, d, :], xTs[:, d, qc * 512:(qc + 1) * 512],
                                     start=(d == 0), stop=(d == ND - 1))
                nc.vector.tensor_copy(kT[:, qc * 512:(qc + 1) * 512], ps2[:])
            for kc in range(KC):
                psv = prefps.tile([128, 128], f32, tag="pv")
                for d in range(ND):
                    nc.tensor.matmul(psv[:], xTs[:, d, kc * 128:(kc + 1) * 128], wv[:, d, :],
                                     start=(d == 0), stop=(d == ND - 1))
                nc.vector.tensor_copy(VA[:, kc * 65:kc * 65 + 64], psv[:, 0:64])
                nc.vector.tensor_copy(VB[:, kc * 65:kc * 65 + 64], psv[:, 64:128])

        # ---- attention + projection, pipelined over (qb, h) ----
        with tc.tile_pool(name="pt", bufs=KC + 2) as ptp, \
             tc.tile_pool(name="srowp", bufs=2) as srp, \
             tc.tile_pool(name="ysb", bufs=4) as yp, \
             tc.tile_pool(name="dramscratch", bufs=2, space="DRAM") as dp, \
             tc.tile_pool(name="stps", bufs=3, space="PSUM") as stps, \
             tc.tile_pool(name="otps", bufs=1, space="PSUM") as otps, \
             tc.tile_pool(name="yps", bufs=1, space="PSUM") as yps:

            for qb in range(NQB):
                q0 = qb * QB
                for h in range(2):
                    hp = 64 * h
                    kTh = kT
                    pts = []
                    for kc in range(KC):
                        st = stps.tile([128, QB], f32, tag="st")
                        for j in range(QC):
                            nc.tensor.matmul(
                                st[:, j * 512:(j + 1) * 512],
                                kT[hp:hp + 64, kc * 128:(kc + 1) * 128],
                                qT[hp:hp + 64, q0 + j * 512:q0 + (j + 1) * 512],
                                start=True, stop=True)
                        pt = ptp.tile([128, QB], bf16, tag="pt")
                        nc.scalar.activation(pt[:], st[:], Exp, scale=0.125)
                        pts.append(pt)
                    Vh = VA if h == 0 else VB
                    OTh = OTa if h == 0 else OTb
                    srow = srp.tile([65, QB], f32, tag="srow")
                    for j in range(QC):
                        po = otps.tile([65, 512], f32, tag="ot")
                        for kc in range(KC):
                            nc.tensor.matmul(po[:], Vh[:, kc * 65:(kc + 1) * 65],
                                             pts[kc][:, j * 512:(j + 1) * 512],
                                             start=(kc == 0), stop=(kc == KC - 1))
                        nc.vector.tensor_copy(
                            OTh[:, q0 + j * 512:q0 + (j + 1) * 512], po[0:64, :])
                        nc.vector.tensor_copy(srow[64:65, j * 512:(j + 1) * 512],
                                              po[64:65, :])
                    # sums -> per-partition layout via DRAM round trip
                    srd = dp.tile([1, QB], f32, tag="srd")
                    nc.sync.dma_start(srd[:], srow[64:65, :])
                    scol = scolA if h == 0 else scolB
                    rcol = rcolA if h == 0 else rcolB
                    nc.sync.dma_start(
                        scol[:, qb * MT:(qb + 1) * MT],
                        srd.rearrange("a (m p) -> (a p) m", p=128))
                    nc.vector.reciprocal(rcol[:, qb * MT:(qb + 1) * MT],
                                         scol[:, qb * MT:(qb + 1) * MT])
                # projection for this query block (both heads ready)
                for m in range(MT):
                    mg = qb * MT + m
                    c0 = q0 + m * 128
                    pa = yps.tile([128, 512], f32, tag="ya")
                    nc.tensor.matmul(pa[:], OTa[:, c0:c0 + 128], wo[:, 0, :],
                                     start=True, stop=True)
                    ya = yp.tile([128, 512], f32, tag="ysa")
                    nc.vector.tensor_scalar_mul(ya[:], pa[:], rcolA[:, mg:mg + 1])
                    pb = yps.tile([128, 512], f32, tag="ya")
                    nc.tensor.matmul(pb[:], OTb[:, c0:c0 + 128], wo[:, 1, :],
                                     start=True, stop=True)
                    yb = yp.tile([128, 512], f32, tag="ysb")
                    nc.vector.tensor_scalar_mul(yb[:], pb[:], rcolB[:, mg:mg + 1])
                    yo = yp.tile([128, 512], f32, tag="yso")
                    nc.vector.tensor_add(yo[:], ya[:], yb[:])
                    nc.sync.dma_start(Y[c0:c0 + 128, :], yo[:])
    nc.compile()
    return nc


_NC_CACHE = {}


def _get_nc(N=SEQ):
    if N not in _NC_CACHE:
        _NC_CACHE[N] = build_nc(N)
    return _NC_CACHE[N]


def kernel(x, Wq, Wk, Wv, Wo, bo):
    x = np.asarray(x, dtype=np.float32)
    Wq = np.asarray(Wq, dtype=np.float32)
    Wk = np.asarray(Wk, dtype=np.float32)
    Wv = np.asarray(Wv, dtype=np.float32)
    Wo = np.asarray(Wo, dtype=np.float32)
    bo = np.asarray(bo, dtype=np.float32)
    Bx, N, Dx = x.shape
    nc = _get_nc(N)
    in_maps = []
    import ml_dtypes
    bfl = ml_dtypes.bfloat16
    xTs = [np.ascontiguousarray(x[b].T).astype(bfl) for b in range(Bx)]
    for c in range(N_CORES):
        b = c // 4
        hA = 2 * (c % 4)
        cols = slice(hA * DH, (hA + 2) * DH)
        in_maps.append({
            "xT": xTs[b],
            "Wq2": np.ascontiguousarray(Wq[:, cols]).astype(bfl),
            "Wk2": np.ascontiguousarray(Wk[:, cols]).astype(bfl),
            "Wv2": np.ascontiguousarray(Wv[:, cols]).astype(bfl),
            "Wo2": np.ascontiguousarray(Wo[cols, :]).astype(bfl),
        })
    res = bass_utils.run_bass_kernel_spmd(nc, in_maps, core_ids=list(range(N_CORES)))
    out = np.zeros((Bx, N, Dx), dtype=np.float32)
    for c in range(N_CORES):
        out[c // 4] += res.results[c]["Y"]
    out += bo
    return out
